# revision 8
# baseline (speedup 1.0000x reference)
"""Relative-position attention (BERT relative_key_query) on 8 trn2 NeuronCores.

Sharding: head-parallel. Each core owns 2 of 16 heads (all 4 batches):
 - projections computed per-core with tensor-parallel weight slices,
 - attention (with both relative-position score terms) per (batch, head),
 - partial output projection per core; host sums the 8 partials (+bo).

v2 pipeline (vs v1): no PE transposes for term1. Both rel-term bands are
computed per l/r-tile into a combined-head [128, 2, 1152] band, evacuated
f32->f16, then:
 - term1 (Q x Erev): diagonal-DMA skew -> [128, 2, 1024] stage -> XBAR
   DMA-transpose straight into rel[:, :, :, l-tile] (r-major),
 - term2 (K x E): diagonal-DMA skew -> stage -> gpsimd in-place add into
   rel[:, :, s, :].
Scores are assembled transposed in PSUM: qk matmul pair (2-head row-packed,
concurrent PE row-groups) + identity-matmul accumulate of the rel slice.
Softmax denominators ride the AV matmul as a ones-column (M=65).
"""

import numpy as np

B = 4
L = 1024
HIDDEN = 1024
HEADS = 16
HEAD = 64
MAXLEN = 1024
NCORES = 8
HPC = HEADS // NCORES          # heads per core = 2
DH2 = HPC * HEAD               # per-core projection width = 128
NT = L // 128                  # 8 row tiles
BAND = 1152                    # skew band width (1151 rounded up)
E2 = 2048                      # padded distance-table width

_CACHE = {}


def _build_nc():
    import concourse.mybir as mybir
    import concourse.tile as tile
    from concourse import bacc
    from concourse.ap import AP

    f32 = mybir.dt.float32
    f16 = mybir.dt.float16
    EXP = mybir.ActivationFunctionType.Exp
    ADD = mybir.AluOpType.add

    nc = bacc.Bacc("TRN2", target_bir_lowering=False, debug=False,
                   num_devices=NCORES)

    qT_d = nc.dram_tensor("qT", (B, HIDDEN, L), f16, kind="ExternalInput")
    kT_d = nc.dram_tensor("kT", (B, HIDDEN, L), f16, kind="ExternalInput")
    vT_d = nc.dram_tensor("vT", (B, HIDDEN, L), f16, kind="ExternalInput")
    wq_d = nc.dram_tensor("wq", (HIDDEN, DH2), f16, kind="ExternalInput")
    wk_d = nc.dram_tensor("wk", (HIDDEN, DH2), f16, kind="ExternalInput")
    wv_d = nc.dram_tensor("wv", (HIDDEN, DH2), f16, kind="ExternalInput")
    wo_d = nc.dram_tensor("wo", (DH2, HIDDEN), f16, kind="ExternalInput")
    bqc_d = nc.dram_tensor("bqc", (DH2, 1), f32, kind="ExternalInput")
    bkc_d = nc.dram_tensor("bkc", (DH2, 1), f32, kind="ExternalInput")
    bvb_d = nc.dram_tensor("bvb", (128, DH2), f32, kind="ExternalInput")
    eT_d = nc.dram_tensor("eT", (128, E2), f16, kind="ExternalInput")
    erevT_d = nc.dram_tensor("erevT", (128, E2), f16, kind="ExternalInput")
    ident_d = nc.dram_tensor("ident", (128, 128), f16, kind="ExternalInput")
    out_d = nc.dram_tensor("out", (B, L, HIDDEN), f16, kind="ExternalOutput")

    def diag2(t, n):
        # t: [128, 2, BAND] tile ap. out[p, h, j] = t[p, h, 127 - p + j]
        return AP(tensor=t.tensor, offset=t.offset + 127,
                  ap=[[2 * BAND - 1, 128], [BAND, 2], [1, n]])

    with tile.TileContext(nc) as tc:
        import contextlib
        with contextlib.ExitStack() as ctx:
            const = ctx.enter_context(tc.tile_pool(name="const", bufs=1))
            actp = ctx.enter_context(tc.tile_pool(name="act", bufs=4))
            vactp = ctx.enter_context(tc.tile_pool(name="vact", bufs=8))
            projp = ctx.enter_context(tc.tile_pool(name="proj", bufs=5))
            vaugp = ctx.enter_context(tc.tile_pool(name="vaug", bufs=2 * NT))
            bbandp = ctx.enter_context(tc.tile_pool(name="bband", bufs=4))
            stagep = ctx.enter_context(tc.tile_pool(name="stage", bufs=7))
            relp = ctx.enter_context(tc.tile_pool(name="rel", bufs=2))
            epp = ctx.enter_context(tc.tile_pool(name="ep", bufs=8))
            ctxp = ctx.enter_context(tc.tile_pool(name="ctx", bufs=2))
            ctxrawp = ctx.enter_context(tc.tile_pool(name="ctxraw", bufs=2))
            smallp = ctx.enter_context(tc.tile_pool(name="small", bufs=2))
            recipbp = ctx.enter_context(tc.tile_pool(name="recipb", bufs=2))
            outp = ctx.enter_context(tc.tile_pool(name="outp", bufs=3))
            ps_main = ctx.enter_context(tc.tile_pool(name="psmain", bufs=4, space="PSUM"))
            ps_ctx = ctx.enter_context(tc.tile_pool(name="psctx", bufs=2, space="PSUM"))

            # ---- constants ----
            wq_sb = const.tile([128, 8, DH2], f16, tag="wq")
            nc.sync.dma_start(wq_sb[:], wq_d[:].rearrange("(c p) d -> p c d", p=128))
            wk_sb = const.tile([128, 8, DH2], f16, tag="wk")
            nc.sync.dma_start(wk_sb[:], wk_d[:].rearrange("(c p) d -> p c d", p=128))
            wv_sb = const.tile([128, 8, DH2], f16, tag="wv")
            nc.sync.dma_start(wv_sb[:], wv_d[:].rearrange("(c p) d -> p c d", p=128))
            wo_sb = const.tile([128, HIDDEN], f16, tag="wo")
            nc.sync.dma_start(wo_sb[:], wo_d[:])
            bqc_sb = const.tile([DH2, 1], f32, tag="bqc")
            nc.sync.dma_start(bqc_sb[:], bqc_d[:])
            bkc_sb = const.tile([DH2, 1], f32, tag="bkc")
            nc.sync.dma_start(bkc_sb[:], bkc_d[:])
            bvb_sb = const.tile([128, DH2], f32, tag="bvb")
            nc.sync.dma_start(bvb_sb[:], bvb_d[:])
            eT_sb = const.tile([128, E2], f16, tag="eT")
            nc.sync.dma_start(eT_sb[:], eT_d[:])
            erevT_sb = const.tile([128, E2], f16, tag="erevT")
            nc.sync.dma_start(erevT_sb[:], erevT_d[:])
            ident_sb = const.tile([128, 128], f16, tag="ident")
            nc.sync.dma_start(ident_sb[:], ident_d[:])

            pending = []

            def emit_tail(pb, pcps, pctx2):
                for h in range(HPC):
                    craw = ctxrawp.tile([65, L], f32, tag="ctxraw")
                    nc.scalar.copy(craw[:], pcps[h][0:65, :])
                    den = smallp.tile([1, L], f32, tag="den")
                    nc.scalar.copy(den[:], pcps[h][64:65, :])
                    recipb = recipbp.tile([64, L], f32, tag="recipb")
                    nc.gpsimd.partition_broadcast(recipb[:], den[:])
                    nc.vector.reciprocal_approx_fast(out=recipb[:], in_=recipb[:])
                    nc.vector.tensor_mul(pctx2[64 * h:64 * h + 64, :], craw[0:64, :],
                                         recipb[:])
                for t in range(NT):
                    ob = outp.tile([128, L], f16, tag="ob")
                    for n in (0, 1):
                        op = ps_main.tile([128, 512], f32, tag="ps")
                        nc.tensor.matmul(op[:], pctx2[:, 128 * t:128 * t + 128],
                                         wo_sb[:, 512 * n:512 * n + 512],
                                         start=True, stop=True)
                        nc.vector.tensor_copy(ob[:, 512 * n:512 * n + 512], op[:])
                    nc.sync.dma_start(out_d[pb, 128 * t:128 * t + 128, :], ob[:])

            for b in range(B):
                # ---- projections: q, k head-dim-major; v interleaved (r-major) ----
                vts = []
                for c in range(8):
                    a = vactp.tile([128, L], f16, tag="vact")
                    nc.sync.dma_start(a[:], vT_d[b, 128 * c:128 * c + 128, :])
                    vts.append(a)

                vaugs = []

                def v_mms():
                    # yields after each v matmul; produces vaugs[s]
                    for s in range(NT):
                        vp = ps_main.tile([128, 512], f32, tag="ps")
                        for c in range(8):
                            nc.tensor.matmul(vp[:, 0:DH2],
                                             vts[c][:, 128 * s:128 * s + 128],
                                             wv_sb[:, c, :], start=(c == 0),
                                             stop=(c == 7))
                            yield
                        va = vaugp.tile([128, 2 * (HEAD + 1)], f16, tag="vaug")
                        for h in range(HPC):
                            nc.vector.scalar_tensor_tensor(
                                va[:, 65 * h:65 * h + 64], vp[:, 64 * h:64 * h + 64],
                                1.0, bvb_sb[:, 64 * h:64 * h + 64],
                                mybir.AluOpType.mult, mybir.AluOpType.add)
                            nc.vector.memset(va[:, 65 * h + 64:65 * h + 65], 1.0)
                        vaugs.append(va)

                qkt = []
                for wsb, bcol, src in ((wq_sb, bqc_sb, qT_d), (wk_sb, bkc_sb, kT_d)):
                    dst = projp.tile([128, L], f16, tag="qk")
                    half = [ps_main.tile([128, 512], f32, tag="ps", name="half")
                            for _ in (0, 1)]
                    for c in range(8):
                        a = actp.tile([128, L], f16, tag="act")
                        nc.sync.dma_start(a[:], src[b, 128 * c:128 * c + 128, :])
                        for n in (0, 1):
                            nc.tensor.matmul(half[n][:], wsb[:, c, :],
                                             a[:, 512 * n:512 * n + 512],
                                             start=(c == 0), stop=(c == 7))
                    for n in (0, 1):
                        nc.vector.tensor_scalar(dst[:, 512 * n:512 * n + 512],
                                                half[n][:], bcol[:], None,
                                                mybir.AluOpType.add)
                    qkt.append(dst)
                for _ in v_mms():
                    pass
                QT2, KT2 = qkt

                if pending:
                    pb, pcps, pctx2 = pending.pop(0)
                    emit_tail(pb, pcps, pctx2)

                # ---- phase 1: banded QDrev / KD, skew + transpose into rel ----
                rel = relp.tile([128, 2, NT, L], f16, tag="rel")
                for term in (0, 1):
                    stat = QT2 if term == 0 else KT2
                    emb = erevT_sb if term == 0 else eT_sb
                    for t in range(NT):
                        base = 896 - 128 * t
                        bb = bbandp.tile([128, 2, BAND], f16, tag="bband")
                        for o, w in ((0, 512), (512, 512), (1024, 128)):
                            qps = []
                            for h in range(HPC):
                                hs = slice(64 * h, 64 * h + 64)
                                qp = ps_main.tile([128, 512], f32, tag="ps")
                                nc.tensor.matmul(qp[:, 0:w],
                                                 stat[hs, 128 * t:128 * t + 128],
                                                 emb[hs, base + o:base + o + w],
                                                 start=True, stop=True)
                                qps.append(qp)
                            for h in range(HPC):
                                if term == 0:
                                    nc.vector.tensor_copy(bb[:, h, o:o + w],
                                                          qps[h][:, 0:w])
                                else:
                                    nc.scalar.copy(bb[:, h, o:o + w], qps[h][:, 0:w])
                        stg = stagep.tile([128, 2, L], f16, tag="stage")
                        nc.sync.dma_start(stg[:], diag2(bb[:], L))
                        if term == 0:
                            nc.scalar.dma_start_transpose(
                                rel[:, :, :, 128 * t:128 * t + 128], stg[:])
                        else:
                            nc.gpsimd.tensor_tensor(rel[:, :, t, :], rel[:, :, t, :],
                                                    stg[:], ADD)

                # ---- phase 2: scores (transposed), softmax, AV ----
                ctx2 = ctxp.tile([128, L], f16, tag="ctx2")
                cps = [ps_ctx.tile([128, L], f32, tag="cps", name="cps")
                       for _ in range(HPC)]
                for s in range(NT):
                    for n in (0, 1):
                        sts = []
                        for h in range(HPC):
                            hs = slice(64 * h, 64 * h + 64)
                            st = ps_main.tile([128, 512], f32, tag="ps")
                            nc.tensor.matmul(st[:], KT2[hs, 128 * s:128 * s + 128],
                                             QT2[hs, 512 * n:512 * n + 512],
                                             start=True, stop=False)
                            sts.append(st)
                        for h in range(HPC):
                            st = sts[h]
                            nc.tensor.matmul(st[:], ident_sb[:],
                                             rel[:, h, s, 512 * n:512 * n + 512],
                                             start=False, stop=True)
                            ep = epp.tile([128, 512], f16, tag="ep")
                            nc.scalar.activation(ep[:], st[:], EXP, scale=0.125)
                            nc.tensor.matmul(cps[h][0:65, 512 * n:512 * n + 512],
                                             vaugs[s][:, 65 * h:65 * h + 65], ep[:],
                                             start=(s == 0), stop=(s == NT - 1))
                pending.append((b, cps, ctx2))

            while pending:
                pb, pcps, pctx2 = pending.pop(0)
                emit_tail(pb, pcps, pctx2)

    nc.compile()
    return nc


def _get_nc():
    if "nc" not in _CACHE:
        _CACHE["nc"] = _build_nc()
    return _CACHE["nc"]


def _prep_in_maps(query, key, value, Wq, bq, Wk, bk, Wv, bv, Wo, bo, dist_emb):
    f32, f16 = np.float32, np.float16
    qT = np.ascontiguousarray(np.transpose(np.asarray(query, f32), (0, 2, 1)).astype(f16))
    kT = np.ascontiguousarray(np.transpose(np.asarray(key, f32), (0, 2, 1)).astype(f16))
    vT = np.ascontiguousarray(np.transpose(np.asarray(value, f32), (0, 2, 1)).astype(f16))
    E = np.asarray(dist_emb, f32)
    eT = np.zeros((128, E2), f16)
    eT[0:64, :2 * MAXLEN - 1] = E.T.astype(f16)
    eT[64:128] = eT[0:64]
    erevT = np.zeros((128, E2), f16)
    erevT[0:64, :2 * MAXLEN - 1] = E[::-1].T.astype(f16)
    erevT[64:128] = erevT[0:64]
    ident = np.eye(128, dtype=f16)
    in_maps = []
    for c in range(NCORES):
        sl = slice(DH2 * c, DH2 * (c + 1))
        in_maps.append({
            "qT": qT, "kT": kT, "vT": vT,
            "wq": np.ascontiguousarray(np.asarray(Wq, f32)[sl, :].T.astype(f16)),
            "wk": np.ascontiguousarray(np.asarray(Wk, f32)[sl, :].T.astype(f16)),
            "wv": np.ascontiguousarray(np.asarray(Wv, f32)[sl, :].T.astype(f16)),
            "wo": np.ascontiguousarray(np.asarray(Wo, f32)[:, sl].T.astype(f16)),
            "bqc": np.asarray(bq, f32)[sl].reshape(DH2, 1),
            "bkc": np.asarray(bk, f32)[sl].reshape(DH2, 1),
            "bvb": np.tile(np.asarray(bv, f32)[sl].reshape(1, DH2), (128, 1)),
            "eT": eT, "erevT": erevT, "ident": ident,
        })
    return in_maps


def run(inputs, trace=False):
    from concourse.bass_utils import run_bass_kernel_spmd
    nc = _get_nc()
    in_maps = _prep_in_maps(**inputs)
    res = run_bass_kernel_spmd(nc, in_maps, core_ids=list(range(NCORES)),
                               trace=trace)
    out = np.zeros((B, L, HIDDEN), np.float32)
    for r in res.results:
        out += r["out"].astype(np.float32)
    out += np.asarray(inputs["bo"], np.float32)[None, None, :]
    return out, res


def kernel(**inputs):
    out, _ = run(inputs, trace=False)
    return out


# revision 11
# speedup vs baseline: 1.1461x; 1.1461x over previous
"""Relative-position attention (BERT relative_key_query) on 8 trn2 NeuronCores.

Sharding: head-parallel. Each core owns 2 of 16 heads (all 4 batches):
 - projections computed per-core with tensor-parallel weight slices,
 - attention (with both relative-position score terms) per (batch, head),
 - partial output projection per core; host sums the 8 partials (+bo).

v3 pipeline: no PE transposes. Per batch:
 - front: input DMAs + q/k/v projections + banded rel-position terms.
   term1 (Q x Erev): band -> evac f16 -> diagonal-DMA skew -> XBAR
   DMA-transpose into rel[:, h, s, l] (r-major).
   term2 (K x E): band -> evac -> skew -> DVE in-place add into rel.
 - phase2: transposed scores in PSUM = packed qk matmul pair + identity-
   matmul accumulate of rel slice; exp on ACT; AV with ones-column (M=65).
 - tail: normalize by denominator column, output projection, DMA out.
Emission is software-pipelined front(b+1) -> tail(b-1) -> phase2(b) so the
band/skew/transpose chain of b+1 hides under phase2(b).
"""

import numpy as np

B = 4
L = 1024
HIDDEN = 1024
HEADS = 16
HEAD = 64
MAXLEN = 1024
NCORES = 8
HPC = HEADS // NCORES          # heads per core = 2
DH2 = HPC * HEAD               # per-core projection width = 128
NT = L // 128                  # 8 row tiles
BAND = 1152                    # skew band width (1151 rounded up)
E2 = 2048                      # padded distance-table width

_CACHE = {}


def _build_nc():
    import concourse.mybir as mybir
    import concourse.tile as tile
    from concourse import bacc
    from concourse.ap import AP

    f32 = mybir.dt.float32
    f16 = mybir.dt.float16
    EXP = mybir.ActivationFunctionType.Exp
    ADD = mybir.AluOpType.add

    nc = bacc.Bacc("TRN2", target_bir_lowering=False, debug=False,
                   num_devices=NCORES)

    qT_d = nc.dram_tensor("qT", (B, HIDDEN, L), f16, kind="ExternalInput")
    kT_d = nc.dram_tensor("kT", (B, HIDDEN, L), f16, kind="ExternalInput")
    vT_d = nc.dram_tensor("vT", (B, HIDDEN, L), f16, kind="ExternalInput")
    wq_d = nc.dram_tensor("wq", (HIDDEN, DH2), f16, kind="ExternalInput")
    wk_d = nc.dram_tensor("wk", (HIDDEN, DH2), f16, kind="ExternalInput")
    wv_d = nc.dram_tensor("wv", (HIDDEN, DH2), f16, kind="ExternalInput")
    wo_d = nc.dram_tensor("wo", (DH2, HIDDEN), f16, kind="ExternalInput")
    bqc_d = nc.dram_tensor("bqc", (DH2, 1), f32, kind="ExternalInput")
    bkc_d = nc.dram_tensor("bkc", (DH2, 1), f32, kind="ExternalInput")
    bvb_d = nc.dram_tensor("bvb", (128, DH2), f32, kind="ExternalInput")
    eT_d = nc.dram_tensor("eT", (128, E2), f16, kind="ExternalInput")
    erevT_d = nc.dram_tensor("erevT", (128, E2), f16, kind="ExternalInput")
    ident_d = nc.dram_tensor("ident", (128, 128), f16, kind="ExternalInput")
    out_d = nc.dram_tensor("out", (B, L, HIDDEN), f16, kind="ExternalOutput")

    def diag2(t, n):
        # t: [128, 2, BAND] tile ap. out[p, h, j] = t[p, h, 127 - p + j]
        return AP(tensor=t.tensor, offset=t.offset + 127,
                  ap=[[2 * BAND - 1, 128], [BAND, 2], [1, n]])

    with tile.TileContext(nc) as tc:
        import contextlib
        with contextlib.ExitStack() as ctx:
            const = ctx.enter_context(tc.tile_pool(name="const", bufs=1))
            actp = ctx.enter_context(tc.tile_pool(name="act", bufs=3))
            vactp = ctx.enter_context(tc.tile_pool(name="vact", bufs=4))
            projp = ctx.enter_context(tc.tile_pool(name="proj", bufs=5))
            vaugp = ctx.enter_context(tc.tile_pool(name="vaug", bufs=2 * NT))
            bbandp = ctx.enter_context(tc.tile_pool(name="bband", bufs=4))
            stagep = ctx.enter_context(tc.tile_pool(name="stage", bufs=6))
            relp = ctx.enter_context(tc.tile_pool(name="rel", bufs=2))
            epp = ctx.enter_context(tc.tile_pool(name="ep", bufs=6))
            ctxp = ctx.enter_context(tc.tile_pool(name="ctx", bufs=2))
            ctxrawp = ctx.enter_context(tc.tile_pool(name="ctxraw", bufs=2))
            smallp = ctx.enter_context(tc.tile_pool(name="small", bufs=2))
            recipbp = ctx.enter_context(tc.tile_pool(name="recipb", bufs=2))
            outp = ctx.enter_context(tc.tile_pool(name="outp", bufs=2))
            ps_band = ctx.enter_context(tc.tile_pool(name="psband", bufs=2, space="PSUM"))
            ps_main = ctx.enter_context(tc.tile_pool(name="psmain", bufs=2, space="PSUM"))
            ps_ctx = ctx.enter_context(tc.tile_pool(name="psctx", bufs=2, space="PSUM"))

            # ---- constants ----
            wq_sb = const.tile([128, 8, DH2], f16, tag="wq")
            nc.sync.dma_start(wq_sb[:], wq_d[:].rearrange("(c p) d -> p c d", p=128))
            wk_sb = const.tile([128, 8, DH2], f16, tag="wk")
            nc.sync.dma_start(wk_sb[:], wk_d[:].rearrange("(c p) d -> p c d", p=128))
            wv_sb = const.tile([128, 8, DH2], f16, tag="wv")
            nc.sync.dma_start(wv_sb[:], wv_d[:].rearrange("(c p) d -> p c d", p=128))
            wo_sb = const.tile([128, HIDDEN], f16, tag="wo")
            nc.sync.dma_start(wo_sb[:], wo_d[:])
            bqc_sb = const.tile([DH2, 1], f32, tag="bqc")
            nc.sync.dma_start(bqc_sb[:], bqc_d[:])
            bkc_sb = const.tile([DH2, 1], f32, tag="bkc")
            nc.sync.dma_start(bkc_sb[:], bkc_d[:])
            bvb_sb = const.tile([128, DH2], f32, tag="bvb")
            nc.sync.dma_start(bvb_sb[:], bvb_d[:])
            eT_sb = const.tile([128, E2], f16, tag="eT")
            nc.sync.dma_start(eT_sb[:], eT_d[:])
            erevT_sb = const.tile([128, E2], f16, tag="erevT")
            nc.sync.dma_start(erevT_sb[:], erevT_d[:])
            ident_sb = const.tile([128, 128], f16, tag="ident")
            nc.sync.dma_start(ident_sb[:], ident_d[:])

            def emit_front(b):
                """inputs + projections + phase1 (rel assembly) for batch b."""
                # v input (4 DMAs of 2 c-tiles each)
                vts = []
                for j in range(4):
                    a = vactp.tile([128, 2, L], f16, tag="vact")
                    nc.sync.dma_start(
                        a[:], vT_d[b, 256 * j:256 * j + 256, :]
                        .rearrange("(c p) l -> p c l", p=128))
                    vts.append(a)

                qkt = []
                for wsb, bcol, src in ((wq_sb, bqc_sb, qT_d), (wk_sb, bkc_sb, kT_d)):
                    dst = projp.tile([128, L], f16, tag="qk")
                    half = [ps_band.tile([128, 512], f32, tag="psb", name="half")
                            for _ in (0, 1)]
                    for j in range(4):
                        a = actp.tile([128, 2, L], f16, tag="act")
                        nc.sync.dma_start(
                            a[:], src[b, 256 * j:256 * j + 256, :]
                            .rearrange("(c p) l -> p c l", p=128))
                        for i in (0, 1):
                            c = 2 * j + i
                            for n in (0, 1):
                                nc.tensor.matmul(half[n][:], wsb[:, c, :],
                                                 a[:, i, 512 * n:512 * n + 512],
                                                 start=(c == 0), stop=(c == 7))
                    for n in (0, 1):
                        nc.vector.tensor_scalar(dst[:, 512 * n:512 * n + 512],
                                                half[n][:], bcol[:], None,
                                                mybir.AluOpType.add)
                    qkt.append(dst)
                QT2, KT2 = qkt

                vaugs = []
                for s in range(NT):
                    vp = ps_main.tile([128, 512], f32, tag="ps")
                    for c in range(8):
                        nc.tensor.matmul(vp[:, 0:DH2],
                                         vts[c // 2][:, c % 2, 128 * s:128 * s + 128],
                                         wv_sb[:, c, :], start=(c == 0),
                                         stop=(c == 7))
                    va = vaugp.tile([128, 2 * (HEAD + 1)], f16, tag="vaug")
                    for h in range(HPC):
                        nc.vector.scalar_tensor_tensor(
                            va[:, 65 * h:65 * h + 64], vp[:, 64 * h:64 * h + 64],
                            1.0, bvb_sb[:, 64 * h:64 * h + 64],
                            mybir.AluOpType.mult, mybir.AluOpType.add)
                        nc.vector.memset(va[:, 65 * h + 64:65 * h + 65], 1.0)
                    vaugs.append(va)

                # phase 1: bands -> evac -> skew -> transpose/merge into rel
                rel = relp.tile([128, 2, NT, L], f16, tag="rel")
                for term in (0, 1):
                    stat = QT2 if term == 0 else KT2
                    emb = erevT_sb if term == 0 else eT_sb
                    for t in range(NT):
                        base = 896 - 128 * t
                        bb = bbandp.tile([128, 2, BAND], f16, tag="bband")
                        for ci, (o, w) in enumerate(((0, 512), (512, 512),
                                                     (1024, 128))):
                            qps = []
                            for h in range(HPC):
                                hs = slice(64 * h, 64 * h + 64)
                                qp = ps_band.tile([128, 512], f32, tag="psb")
                                nc.tensor.matmul(qp[:, 0:w],
                                                 stat[hs, 128 * t:128 * t + 128],
                                                 emb[hs, base + o:base + o + w],
                                                 start=True, stop=True)
                                qps.append(qp)
                            for h in range(HPC):
                                if ci == 1:
                                    nc.scalar.copy(bb[:, h, o:o + w], qps[h][:, 0:w])
                                else:
                                    nc.vector.tensor_copy(bb[:, h, o:o + w],
                                                          qps[h][:, 0:w])
                        stg = stagep.tile([128, 2, L], f16, tag="stage")
                        nc.sync.dma_start(stg[:], diag2(bb[:], L))
                        if term == 0:
                            nc.scalar.dma_start_transpose(
                                rel[:, :, :, 128 * t:128 * t + 128], stg[:])
                        else:
                            nc.vector.tensor_tensor(rel[:, :, t, :],
                                                    rel[:, :, t, :], stg[:], ADD)
                return QT2, KT2, vaugs, rel

            def emit_phase2(b, QT2, KT2, vaugs, rel):
                ctx2 = ctxp.tile([128, L], f16, tag="ctx2")
                cps = [ps_ctx.tile([128, L], f32, tag="cps", name="cps")
                       for _ in range(HPC)]
                for s in range(NT):
                    for n in (0, 1):
                        sts = []
                        for h in range(HPC):
                            hs = slice(64 * h, 64 * h + 64)
                            st = ps_main.tile([128, 512], f32, tag="ps")
                            nc.tensor.matmul(st[:], KT2[hs, 128 * s:128 * s + 128],
                                             QT2[hs, 512 * n:512 * n + 512],
                                             start=True, stop=False)
                            sts.append(st)
                        for h in range(HPC):
                            st = sts[h]
                            nc.tensor.matmul(st[:], ident_sb[:],
                                             rel[:, h, s, 512 * n:512 * n + 512],
                                             start=False, stop=True)
                            ep = epp.tile([128, 512], f16, tag="ep")
                            nc.scalar.activation(ep[:], st[:], EXP, scale=0.125)
                            nc.tensor.matmul(cps[h][0:65, 512 * n:512 * n + 512],
                                             vaugs[s][:, 65 * h:65 * h + 65], ep[:],
                                             start=(s == 0), stop=(s == NT - 1))
                return cps, ctx2

            def emit_tail(pb, pcps, pctx2):
                for h in range(HPC):
                    craw = ctxrawp.tile([65, L], f32, tag="ctxraw")
                    nc.scalar.copy(craw[:], pcps[h][0:65, :])
                    den = smallp.tile([1, L], f32, tag="den")
                    nc.scalar.copy(den[:], pcps[h][64:65, :])
                    recipb = recipbp.tile([64, L], f32, tag="recipb")
                    nc.gpsimd.partition_broadcast(recipb[:], den[:])
                    nc.vector.reciprocal_approx_fast(out=recipb[:], in_=recipb[:])
                    nc.vector.tensor_mul(pctx2[64 * h:64 * h + 64, :], craw[0:64, :],
                                         recipb[:])
                for tt in range(4):
                    ob = outp.tile([128, 2, L], f16, tag="ob")
                    for i in (0, 1):
                        t = 2 * tt + i
                        for n in (0, 1):
                            op = ps_main.tile([128, 512], f32, tag="ps")
                            nc.tensor.matmul(op[:], pctx2[:, 128 * t:128 * t + 128],
                                             wo_sb[:, 512 * n:512 * n + 512],
                                             start=True, stop=True)
                            nc.vector.tensor_copy(ob[:, i, 512 * n:512 * n + 512],
                                                  op[:])
                    nc.sync.dma_start(
                        out_d[pb, 256 * tt:256 * tt + 256, :]
                        .rearrange("(c p) l -> p c l", p=128), ob[:])

            staged = {0: emit_front(0)}
            pending = []
            for b in range(B):
                if b + 1 < B:
                    staged[b + 1] = emit_front(b + 1)
                if pending:
                    emit_tail(*pending.pop(0))
                cps, ctx2 = emit_phase2(b, *staged.pop(b))
                pending.append((b, cps, ctx2))
            while pending:
                emit_tail(*pending.pop(0))

    nc.compile()
    return nc


def _get_nc():
    if "nc" not in _CACHE:
        _CACHE["nc"] = _build_nc()
    return _CACHE["nc"]


def _prep_in_maps(query, key, value, Wq, bq, Wk, bk, Wv, bv, Wo, bo, dist_emb):
    f32, f16 = np.float32, np.float16
    qT = np.ascontiguousarray(np.transpose(np.asarray(query, f32), (0, 2, 1)).astype(f16))
    kT = np.ascontiguousarray(np.transpose(np.asarray(key, f32), (0, 2, 1)).astype(f16))
    vT = np.ascontiguousarray(np.transpose(np.asarray(value, f32), (0, 2, 1)).astype(f16))
    E = np.asarray(dist_emb, f32)
    eT = np.zeros((128, E2), f16)
    eT[0:64, :2 * MAXLEN - 1] = E.T.astype(f16)
    eT[64:128] = eT[0:64]
    erevT = np.zeros((128, E2), f16)
    erevT[0:64, :2 * MAXLEN - 1] = E[::-1].T.astype(f16)
    erevT[64:128] = erevT[0:64]
    ident = np.eye(128, dtype=f16)
    in_maps = []
    for c in range(NCORES):
        sl = slice(DH2 * c, DH2 * (c + 1))
        in_maps.append({
            "qT": qT, "kT": kT, "vT": vT,
            "wq": np.ascontiguousarray(np.asarray(Wq, f32)[sl, :].T.astype(f16)),
            "wk": np.ascontiguousarray(np.asarray(Wk, f32)[sl, :].T.astype(f16)),
            "wv": np.ascontiguousarray(np.asarray(Wv, f32)[sl, :].T.astype(f16)),
            "wo": np.ascontiguousarray(np.asarray(Wo, f32)[:, sl].T.astype(f16)),
            "bqc": np.asarray(bq, f32)[sl].reshape(DH2, 1),
            "bkc": np.asarray(bk, f32)[sl].reshape(DH2, 1),
            "bvb": np.tile(np.asarray(bv, f32)[sl].reshape(1, DH2), (128, 1)),
            "eT": eT, "erevT": erevT, "ident": ident,
        })
    return in_maps


def run(inputs, trace=False):
    from concourse.bass_utils import run_bass_kernel_spmd
    nc = _get_nc()
    in_maps = _prep_in_maps(**inputs)
    res = run_bass_kernel_spmd(nc, in_maps, core_ids=list(range(NCORES)),
                               trace=trace)
    out = np.zeros((B, L, HIDDEN), np.float32)
    for r in res.results:
        out += r["out"].astype(np.float32)
    out += np.asarray(inputs["bo"], np.float32)[None, None, :]
    return out, res


def kernel(**inputs):
    out, _ = run(inputs, trace=False)
    return out
